# revision 1
# baseline (speedup 1.0000x reference)
"""DRConv (dynamic region-aware conv) Trainium2 kernel.

Math (per batch b, all on device):
  x_se  = 0.25*sigmoid(routing_w @ mean_hw(x) + routing_b)           # [G*T]
  Z_t   = conv3x3(x, template_t)       for t in 0..T-1               # [O, H, W]
  U     = [x_se.T | 1] contracted with exp(Alpha) over g             # [T+1, P]
  out   = (sum_t Z_t * U_t) / U_T  + bias                            # [O, H, W]
which equals the reference
  out = einsum('boghw,bghw->bohw', einsum('bokg,bkhw->boghw', w, patches),
               softmax(Alpha)) + bias
because w = blend(x_se, templates) commutes through the conv: the blend
weights x_se[g,t] and the softmax probs both act per (g, pixel), so the
G-sum and T-sum exchange with the K-contraction.

Sharding: data-parallel over batch B=8, one batch element per NeuronCore.
Templates/routing weights replicated. No collectives.

Device layout (per core):
  pixels live in a 58x57 plane: one pad row top/bottom, ONE pad column
  (a right-pad column doubles as the left neighbor of the next row's
  x=0 pixel, so 57-wide rows give correct 3x3 zero padding);
  pf = (y+1)*57 + x for image pixel (y, x).
  conv = 9 shifted matmuls accumulating in PSUM:
    Z[px, (t,o)] += x[c, base+px+delta(i,j)].T @ tmpl[c, (t,o)]
  pixel tiles are the stationary operand (128 px per matmul), so the
  per-pixel softmax mixing becomes per-partition scalar_tensor_tensor ops,
  and the final [px, o] -> [o, px] flip is a PE transpose.
"""

import ml_dtypes
import numpy as np

import concourse.bass as bass
import concourse.mybir as mybir
from concourse import bacc, masks
from concourse.tile import TileContext
from concourse.bass_utils import run_bass_kernel_spmd

# problem constants
C = 128          # in channels
O = 128          # out channels
H = W = 56
G = 8            # groups
T = 8            # num weight templates
WP = 57          # padded row width (one shared pad column)
HPAD = 58        # one pad row top and bottom
NPIX = HPAD * WP  # 3306
GUARD = 64       # front guard in the x buffer for negative conv shifts
OFREE = 3328     # 26*128 >= NPIX
PT0 = WP         # first pixel-tile starts at padded row 1
NT = 25          # 25 tiles of 128 px cover pf [57, 3257) > last valid 3247
NCORES = 8

_cache = {}


def _delta(ij):
    i, j = divmod(ij, 3)
    return (i - 1) * WP + (j - 1)


def _build(use_alpha: int):
    f32 = mybir.dt.float32
    bf16 = mybir.dt.bfloat16

    nc = bacc.Bacc("TRN2", target_bir_lowering=False, debug=False,
                   num_devices=NCORES)

    # image ships as bf16 (matmuls are bf16 anyway), split in two row
    # bands so early pixel tiles only wait for the first band
    x0_d = nc.dram_tensor("x0", [C, 31 * W], bf16, kind="ExternalInput")
    x1_d = nc.dram_tensor("x1", [C, 28 * W], bf16, kind="ExternalInput")
    alpha_d = nc.dram_tensor("alpha", [G, H, W], f32, kind="ExternalInput")
    tmpl_d = nc.dram_tensor("tmpl", [9, C, T * O], bf16, kind="ExternalInput")
    rwt_d = nc.dram_tensor("rwt", [C, G * T], f32, kind="ExternalInput")
    rb_d = nc.dram_tensor("rb", [G * T], f32, kind="ExternalInput")
    bias_d = nc.dram_tensor("bias", [O], f32, kind="ExternalInput")
    mask_d = None
    if not use_alpha:
        mask_d = nc.dram_tensor("mask", [H, W], mybir.dt.int32,
                                kind="ExternalInput")
    out_d = nc.dram_tensor("out", [O, OFREE], f32, kind="ExternalOutput")

    with TileContext(nc) as tc:
        with (
            tc.tile_pool(name="big", bufs=1) as big,
            tc.tile_pool(name="consts", bufs=1) as consts,
            tc.tile_pool(name="stage", bufs=3) as stage,
            tc.tile_pool(name="acc", bufs=3) as accp,
            tc.tile_pool(name="upool", bufs=3) as upool,
            tc.tile_pool(name="zps", bufs=3, space="PSUM") as zps,
            tc.tile_pool(name="ups", bufs=1, space="PSUM") as ups,
            tc.tile_pool(name="tps", bufs=1, space="PSUM") as tps,
        ):
            # ---- constants ----
            ident = consts.tile([128, 128], f32)
            masks.make_identity(nc, ident[:])

            # PE warmup: dummy matmuls so HAM un-throttles while the
            # input DMAs stream in (needs only SBUF-resident data)
            warm = tps.tile([128, 128], f32, tag="tp", name="warm")
            for w_i in range(30):
                nc.tensor.matmul(warm[:], lhsT=ident[:], rhs=ident[:])

            bias_rep = consts.tile([128, O], f32)
            nc.sync.dma_start(
                out=bias_rep[:],
                in_=bass.AP(tensor=bias_d, offset=0, ap=[[0, 128], [1, O]]),
            )

            # ---- image band A + routing weights first ----
            xst0 = big.tile([C, 31 * W], bf16)
            nc.sync.dma_start(out=xst0[:], in_=x0_d[:])
            rwt = consts.tile([C, G * T], f32)
            nc.sync.dma_start(out=rwt[:], in_=rwt_d[:])
            rb = consts.tile([G * T, 1], f32)
            nc.sync.dma_start(out=rb[:], in_=rb_d[:])

            # band B DMA too (bf16 bands are small; land them both early)
            XB1 = 29 * WP                  # pf origin of band B buffer
            xst1 = big.tile([C, 28 * W], bf16)
            nc.sync.dma_start(out=xst1[:], in_=x1_d[:])

            # pixel tiles k<=12 read pf [-1, 1779) -> image rows 0..30
            xbf0 = big.tile([C, GUARD + 32 * WP], bf16)
            nc.vector.memset(xbf0[:], 0.0)
            v = xbf0[:, GUARD:GUARD + 32 * WP].rearrange(
                "c (h w) -> c h w", w=WP)
            nc.vector.tensor_copy(
                v[:, 1:32, 0:W], xst0[:].rearrange("c (h w) -> c h w", w=W))

            # ---- templates ----
            tbf = []
            for ij in range(9):
                tb = big.tile([C, T * O], bf16, name=f"tbf{ij}")
                nc.sync.dma_start(out=tb[:], in_=tmpl_d[ij])
                tbf.append(tb)

            # ---- image band B plane: k>=13 read pf [1663, 3315) ----
            xbf1 = big.tile([C, 30 * WP], bf16)
            nc.gpsimd.memset(xbf1[:], 0.0)
            v = xbf1[:, 0:30 * WP].rearrange("c (h w) -> c h w", w=WP)
            nc.gpsimd.tensor_copy(
                v[:, 0:28, 0:W], xst1[:].rearrange("c (h w) -> c h w", w=W))

            # ---- routing: GAP -> fc -> sigmoid (start ASAP) ----
            xsum = consts.tile([C, 1], f32)
            xsum0 = consts.tile([C, 1], f32)
            nc.vector.tensor_reduce(
                out=xsum0[:], in_=xst0[:],
                axis=mybir.AxisListType.X, op=mybir.AluOpType.add)
            nc.vector.tensor_reduce(
                out=xsum[:], in_=xst1[:, 3 * W:],
                axis=mybir.AxisListType.X, op=mybir.AluOpType.add)
            nc.vector.tensor_add(xsum[:], xsum[:], xsum0[:])

            zr = ups.tile([G * T, 1], f32, tag="up")
            nc.tensor.matmul(zr[:], lhsT=rwt[:], rhs=xsum[:])
            # x_se = (2/T)*sigmoid(fc(mean) + rb); mean folded into scale
            xse = consts.tile([G * T, 1], f32)
            nc.scalar.activation(xse[:], zr[:],
                                 mybir.ActivationFunctionType.Sigmoid,
                                 bias=rb[:], scale=1.0 / (H * W))
            xse4 = consts.tile([G * T, 1], bf16)
            nc.vector.tensor_scalar_mul(xse4[:], xse[:], 2.0 / T)

            # lhsT_U [g, T+1]: cols 0..T-1 = x_se[g, t], col T = 1.0
            # (the [64,1] -> [8,8] partition/free reshape is a tiny DMA)
            lhsu = consts.tile([G, T + 1], bf16)
            nc.vector.memset(lhsu[:, T:T + 1], 1.0)
            nc.sync.dma_start(out=lhsu[:, 0:T], in_=xse4[:])

            # ---- routing probability numerators ----
            ea = big.tile([G, OFREE], bf16)
            nc.gpsimd.memset(ea[:], 1.0)
            ea_core = ea[:, 0:NPIX].rearrange("g (h w) -> g h w", w=WP)
            if use_alpha:
                astage = stage.tile([G, H * W], f32, tag="astage")
                nc.sync.dma_start(out=astage[:], in_=alpha_d[:])
                nc.scalar.activation(
                    ea_core[:, 1:57, 0:W],
                    astage[:].rearrange("g (h w) -> g h w", w=W),
                    mybir.ActivationFunctionType.Exp)
            else:
                # hard routing: ea[g, p] = (mask[p] == g)
                mrow = stage.tile([1, H * W], mybir.dt.int32, tag="mrow")
                nc.sync.dma_start(out=mrow[:], in_=mask_d[:])
                mf = stage.tile([1, H * W], f32, tag="mf")
                nc.scalar.copy(mf[:], mrow[:])
                mrep = big.tile([G, H * W], f32)
                for g in range(G):
                    nc.sync.dma_start(out=mrep[g:g + 1, :], in_=mf[:])
                giota = consts.tile([G, 1], f32)
                for g in range(G):
                    nc.vector.memset(giota[g:g + 1, :], float(g))
                nc.vector.tensor_scalar(
                    ea_core[:, 1:57, 0:W],
                    mrep[:].rearrange("g (h w) -> g h w", w=W),
                    giota[:], None, op0=mybir.AluOpType.is_equal)

            # ---- output accumulation plane, 4 window-aligned chunks so
            # stores overlap compute and the tail only waits on the last ----
            OCUT = [0, PT0 + 128 * 7, PT0 + 128 * 13, PT0 + 128 * 19, OFREE]
            outsb = [big.tile([O, OCUT[i + 1] - OCUT[i]], f32,
                              name=f"outsb{i}") for i in range(4)]

            def outsb_slice(lo, n):
                for i in range(4):
                    if lo + n <= OCUT[i + 1]:
                        assert lo >= OCUT[i]
                        return outsb[i][:, lo - OCUT[i]:lo - OCUT[i] + n]
                raise AssertionError(lo)

            # ---- main loop over pixel tiles ----
            for k in range(NT):
                base = PT0 + 128 * k

                up = ups.tile([128, T + 1], f32, tag="up")
                nc.tensor.matmul(up[:], lhsT=ea[:, base:base + 128],
                                 rhs=lhsu[:])
                rcol = upool.tile([128, 1], f32, tag="rcol")
                nc.vector.reciprocal(rcol[:], up[:, T:T + 1])
                usb = upool.tile([128, T], f32, tag="usb")
                nc.vector.tensor_scalar_mul(usb[:], up[:, 0:T], rcol[:])

                zp = [zps.tile([128, 512], f32, tag=f"zp{h}",
                               name=f"zp{h}_{k}")
                      for h in range(2)]
                for ij in range(9):
                    if k <= 12:
                        lo = GUARD + base + _delta(ij)
                        xsl = xbf0[:, lo:lo + 128]
                    else:
                        lo = base - XB1 + _delta(ij)
                        xsl = xbf1[:, lo:lo + 128]
                    for h in range(2):
                        nc.tensor.matmul(
                            zp[h][:],
                            lhsT=xsl,
                            rhs=tbf[ij][:, h * 512:(h + 1) * 512],
                            start=(ij == 0), stop=(ij == 8))

                acc = accp.tile([128, O], f32, tag="acc")
                for t in range(T):
                    h, tq = divmod(t, 4)
                    nc.vector.scalar_tensor_tensor(
                        out=acc[:],
                        in0=zp[h][:, tq * 128:(tq + 1) * 128],
                        scalar=usb[:, t:t + 1],
                        in1=bias_rep[:] if t == 0 else acc[:],
                        op0=mybir.AluOpType.mult,
                        op1=mybir.AluOpType.add)

                tp = tps.tile([128, 128], f32, tag="tp")
                nc.tensor.transpose(tp[:], acc[:], ident[:])
                nc.scalar.copy(outsb_slice(base, 128), tp[:])

            # ---- store padded planes (host strips the padding) ----
            for i in range(4):
                nc.sync.dma_start(out=out_d[:, OCUT[i]:OCUT[i + 1]],
                                  in_=outsb[i][:])

    nc.compile()
    return nc


def _get(use_alpha: int):
    if use_alpha not in _cache:
        _cache[use_alpha] = _build(use_alpha)
    return _cache[use_alpha]


def _in_maps(inp):
    ua = int(np.asarray(inp["use_alpha"]))
    x = np.asarray(inp["inputs"], dtype=np.float32).reshape(
        NCORES, C, H * W).astype(ml_dtypes.bfloat16)
    x0 = np.ascontiguousarray(x[:, :, 0:31 * W])
    x1 = np.ascontiguousarray(x[:, :, 28 * W:])
    Alpha = np.ascontiguousarray(np.asarray(inp["Alpha"], dtype=np.float32))
    # [O*C*3*3, T] -> [(i,j), c, t*O + o]
    tmpl = np.asarray(inp["weight_templates"], dtype=np.float32).reshape(
        O, C, 3, 3, T).transpose(2, 3, 1, 4, 0).reshape(9, C, T * O)
    tmpl = np.ascontiguousarray(tmpl).astype(ml_dtypes.bfloat16)
    rwt = np.ascontiguousarray(
        np.asarray(inp["routing_w"], dtype=np.float32).T)
    rb = np.ascontiguousarray(np.asarray(inp["routing_b"], dtype=np.float32))
    bias = np.ascontiguousarray(np.asarray(inp["bias"], dtype=np.float32))

    in_maps = []
    for b in range(NCORES):
        m = {"x0": x0[b], "x1": x1[b], "alpha": Alpha[b], "tmpl": tmpl,
             "rwt": rwt, "rb": rb, "bias": bias}
        if not ua:
            m["mask"] = np.ascontiguousarray(
                np.asarray(inp["mask"][b], dtype=np.int32))
        in_maps.append(m)
    return in_maps


def kernel(inputs, mask, Alpha, weight_templates, routing_w, routing_b, bias,
           use_alpha):
    ua = int(np.asarray(use_alpha))
    nc = _get(ua)
    in_maps = _in_maps(dict(inputs=inputs, mask=mask, Alpha=Alpha,
                            weight_templates=weight_templates,
                            routing_w=routing_w, routing_b=routing_b,
                            bias=bias, use_alpha=use_alpha))
    res = run_bass_kernel_spmd(nc, in_maps, list(range(NCORES)))
    out = np.stack([res.results[b]["out"] for b in range(NCORES)], axis=0)
    out = out[:, :, :NPIX].reshape(NCORES, O, HPAD, WP)[:, :, 1:57, 0:W]
    return np.ascontiguousarray(out)



# revision 8
# speedup vs baseline: 1.0222x; 1.0222x over previous
"""DRConv (dynamic region-aware conv) Trainium2 kernel, v2.

Math (per batch b, all on device):
  x_se  = 0.25*sigmoid(routing_w @ mean_hw(x) + routing_b)           # [G*T]
  Z_t   = conv3x3(x, template_t)       for t in 0..T-1               # [O, H, W]
  U     = [x_se.T | 1] contracted with exp(Alpha) over g             # [T+1, P]
  out   = (sum_t Z_t * U_t) / U_T  + bias                            # [O, H, W]
which equals the reference because the template blend commutes through
the conv (blend weights and softmax probs both act per (g, pixel)).

Sharding: data-parallel over batch B=8, one batch element per core.

v2 changes vs v1:
  - x ships from host already scattered into the padded 57-pitch
    planes (bf16), so no on-device memset/copy staging.
  - Alpha ships zero-padded into the plane layout; exp(0)=1 supplies
    the pad-pixel softmax denominator for free (single ACT, no memset).
  - bf16 warmup matmuls (8) instead of 30 fp32 ones.
  - input DMA issues spread across Sync/Scalar/GpSimd queues.
  - no PE transpose: each tile's [px, o] accumulator is DMA'd straight
    to DRAM; the [px,o] -> [o,hw] flip happens in the host gather.
"""

import ml_dtypes
import numpy as np

import concourse.bass as bass
import concourse.mybir as mybir
from concourse import bacc
from concourse.tile import TileContext
from concourse.bass_utils import run_bass_kernel_spmd

# problem constants
C = 128          # in channels
O = 128          # out channels
H = W = 56
G = 8            # groups
T = 8            # num weight templates
WP = 57          # padded row width (one shared pad column)
HPAD = 58        # one pad row top and bottom
NPIX = HPAD * WP  # 3306
GUARD = 64       # front guard in the x buffer for negative conv shifts
PT0 = WP         # first pixel-tile starts at padded row 1
NT = 25          # 25 tiles of 128 px cover pf [57, 3257) > last valid 3247
AFREE = 3328     # alpha plane free size (NPIX rounded up)
XB1 = 29 * WP    # pf origin of band-B buffer
NB0 = GUARD + 32 * WP   # band-A buffer cols (pf -GUARD .. 1824)
NB1 = 30 * WP           # band-B buffer cols (pf 1653 .. 3363)
NCORES = 8
NWARM = 8        # bf16 warmup matmuls

_cache = {}


def _delta(ij):
    i, j = divmod(ij, 3)
    return (i - 1) * WP + (j - 1)


def _build(use_alpha: int):
    f32 = mybir.dt.float32
    bf16 = mybir.dt.bfloat16

    nc = bacc.Bacc("TRN2", target_bir_lowering=False, debug=False,
                   num_devices=NCORES)

    x0_d = nc.dram_tensor("x0", [C, NB0], bf16, kind="ExternalInput")
    x1_d = nc.dram_tensor("x1", [C, NB1], bf16, kind="ExternalInput")
    if use_alpha:
        alpha_d = nc.dram_tensor("alpha", [G, AFREE], f32,
                                 kind="ExternalInput")
    else:
        # hard routing: host ships ea = one_hot(mask) directly
        ea_d = nc.dram_tensor("ea", [G, AFREE], bf16, kind="ExternalInput")
    tmpl_d = nc.dram_tensor("tmpl", [9, C, T * O], bf16, kind="ExternalInput")
    rwt_d = nc.dram_tensor("rwt", [C, G * T], f32, kind="ExternalInput")
    rb_d = nc.dram_tensor("rb", [G * T], f32, kind="ExternalInput")
    bias_d = nc.dram_tensor("bias", [O], f32, kind="ExternalInput")
    out_d = nc.dram_tensor("out", [NT, 128, O], f32, kind="ExternalOutput")

    with TileContext(nc) as tc:
        with (
            tc.tile_pool(name="big", bufs=1) as big,
            tc.tile_pool(name="consts", bufs=1) as consts,
            tc.tile_pool(name="acc", bufs=3) as accp,
            tc.tile_pool(name="upool", bufs=3) as upool,
            tc.tile_pool(name="zps", bufs=3, space="PSUM") as zps,
            tc.tile_pool(name="ups", bufs=2, space="PSUM") as ups,
        ):
            # ---- PE warmup on a zeroed bf16 tile while inputs stream ----
            warmz = consts.tile([128, 512], bf16)
            nc.vector.memset(warmz[:], 0.0)
            warm = zps.tile([128, 512], f32, tag="zp0", name="warm")
            for _ in range(NWARM):
                nc.tensor.matmul(warm[:], lhsT=warmz[:, 0:128], rhs=warmz[:])

            # ---- input DMAs, issues spread across engine queues ----
            xbf0 = big.tile([C, NB0], bf16)
            nc.sync.dma_start(out=xbf0[:], in_=x0_d[:])
            xbf1 = big.tile([C, NB1], bf16)
            nc.sync.dma_start(out=xbf1[:], in_=x1_d[:])
            if use_alpha:
                ast = big.tile([G, AFREE], f32)
                nc.sync.dma_start(out=ast[:], in_=alpha_d[:])

            tbf = []
            for ij in range(9):
                tb = big.tile([C, T * O], bf16, name=f"tbf{ij}")
                eng = nc.scalar if ij < 4 else nc.gpsimd
                eng.dma_start(out=tb[:], in_=tmpl_d[ij])
                tbf.append(tb)

            rwt = consts.tile([C, G * T], f32)
            nc.gpsimd.dma_start(out=rwt[:], in_=rwt_d[:])
            rb = consts.tile([G * T, 1], f32)
            nc.gpsimd.dma_start(out=rb[:], in_=rb_d[:])
            bias_rep = consts.tile([128, O], f32)
            nc.scalar.dma_start(
                out=bias_rep[:],
                in_=bass.AP(tensor=bias_d, offset=0, ap=[[0, 128], [1, O]]),
            )

            # ---- routing: GAP -> fc -> sigmoid ----
            # band A sum covers image rows 0..30 (pads/guard are zero);
            # band B slice skips its first 3 rows (28..30, already in A)
            xsum = consts.tile([C, 1], f32)
            xsum0 = consts.tile([C, 1], f32)
            nc.vector.tensor_reduce(
                out=xsum0[:], in_=xbf0[:],
                axis=mybir.AxisListType.X, op=mybir.AluOpType.add)
            nc.vector.tensor_reduce(
                out=xsum[:], in_=xbf1[:, 3 * WP:],
                axis=mybir.AxisListType.X, op=mybir.AluOpType.add)
            nc.vector.tensor_add(xsum[:], xsum[:], xsum0[:])

            zr = ups.tile([G * T, 1], f32, tag="up")
            nc.tensor.matmul(zr[:], lhsT=rwt[:], rhs=xsum[:])
            xse = consts.tile([G * T, 1], f32)
            nc.scalar.activation(xse[:], zr[:],
                                 mybir.ActivationFunctionType.Sigmoid,
                                 bias=rb[:], scale=1.0 / (H * W))
            xse4 = consts.tile([G * T, 1], bf16)
            nc.vector.tensor_scalar_mul(xse4[:], xse[:], 2.0 / T)

            # lhsT_U [g, T+1]: cols 0..T-1 = x_se[g, t], col T = 1.0
            lhsu = consts.tile([G, T + 1], bf16)
            nc.vector.memset(lhsu[:, T:T + 1], 1.0)
            nc.sync.dma_start(out=lhsu[:, 0:T], in_=xse4[:])

            # ---- routing numerators: ea = exp(alpha), pads exp(0)=1 ----
            ea = big.tile([G, AFREE], bf16)
            if use_alpha:
                nc.scalar.activation(ea[:], ast[:],
                                     mybir.ActivationFunctionType.Exp)
            else:
                nc.sync.dma_start(out=ea[:], in_=ea_d[:])

            # ---- main loop over pixel tiles ----
            for k in range(NT):
                base = PT0 + 128 * k

                zp = [zps.tile([128, 512], f32, tag=f"zp{h}",
                               name=f"zp{h}_{k}")
                      for h in range(2)]
                for ij in range(9):
                    if k <= 12:
                        lo = GUARD + base + _delta(ij)
                        xsl = xbf0[:, lo:lo + 128]
                    else:
                        lo = base - XB1 + _delta(ij)
                        xsl = xbf1[:, lo:lo + 128]
                    for h in range(2):
                        nc.tensor.matmul(
                            zp[h][:],
                            lhsT=xsl,
                            rhs=tbf[ij][:, h * 512:(h + 1) * 512],
                            start=(ij == 0), stop=(ij == 8))

                up = ups.tile([128, T + 1], f32, tag="up")
                nc.tensor.matmul(up[:], lhsT=ea[:, base:base + 128],
                                 rhs=lhsu[:])
                rcol = upool.tile([128, 1], f32, tag="rcol")
                nc.vector.reciprocal(rcol[:], up[:, T:T + 1])
                usb = upool.tile([128, T], f32, tag="usb")
                nc.vector.tensor_scalar_mul(usb[:], up[:, 0:T], rcol[:])

                acc = accp.tile([128, O], f32, tag="acc")
                for t in range(T):
                    h, tq = divmod(t, 4)
                    nc.vector.scalar_tensor_tensor(
                        out=acc[:],
                        in0=zp[h][:, tq * 128:(tq + 1) * 128],
                        scalar=usb[:, t:t + 1],
                        in1=bias_rep[:] if t == 0 else acc[:],
                        op0=mybir.AluOpType.mult,
                        op1=mybir.AluOpType.add)

                nc.sync.dma_start(out=out_d[k], in_=acc[:])

    nc.compile()
    return nc


def _get(use_alpha: int):
    if use_alpha not in _cache:
        _cache[use_alpha] = _build(use_alpha)
    return _cache[use_alpha]


def _in_maps(inp):
    ua = int(np.asarray(inp["use_alpha"]))
    x = np.asarray(inp["inputs"], dtype=np.float32).reshape(
        NCORES, C, H, W).astype(ml_dtypes.bfloat16)

    # band A: pf [-GUARD, 1824) = plane rows 0..31 (img rows 0..30)
    xb0 = np.zeros((NCORES, C, NB0), ml_dtypes.bfloat16)
    v0 = xb0[:, :, GUARD:].reshape(NCORES, C, 32, WP)
    v0[:, :, 1:32, 0:W] = x[:, :, 0:31, :]
    # band B: pf [1653, 3363) = plane rows 29..58 (img rows 28..55)
    xb1 = np.zeros((NCORES, C, NB1), ml_dtypes.bfloat16)
    v1 = xb1.reshape(NCORES, C, 30, WP)
    v1[:, :, 0:28, 0:W] = x[:, :, 28:56, :]

    if ua:
        # alpha scattered into the plane; zero pads -> exp=1
        al = np.zeros((NCORES, G, AFREE), np.float32)
        va = al[:, :, 0:NPIX].reshape(NCORES, G, HPAD, WP)
        va[:, :, 1:57, 0:W] = np.asarray(inp["Alpha"], dtype=np.float32)
    else:
        # hard routing: ea = one_hot(mask), pads 1.0 (any nonzero denom)
        mk = np.asarray(inp["mask"]).reshape(NCORES, H, W)
        ea = np.ones((NCORES, G, AFREE), np.float32)
        ve = ea[:, :, 0:NPIX].reshape(NCORES, G, HPAD, WP)
        ve[:, :, 1:57, 0:W] = (
            mk[:, None, :, :] == np.arange(G)[None, :, None, None])
        al = ea.astype(ml_dtypes.bfloat16)

    # [O*C*3*3, T] -> [(i,j), c, t*O + o]
    tmpl = np.asarray(inp["weight_templates"], dtype=np.float32).reshape(
        O, C, 3, 3, T).transpose(2, 3, 1, 4, 0).reshape(9, C, T * O)
    tmpl = np.ascontiguousarray(tmpl).astype(ml_dtypes.bfloat16)
    rwt = np.ascontiguousarray(
        np.asarray(inp["routing_w"], dtype=np.float32).T)
    rb = np.ascontiguousarray(np.asarray(inp["routing_b"], dtype=np.float32))
    bias = np.ascontiguousarray(np.asarray(inp["bias"], dtype=np.float32))

    akey = "alpha" if ua else "ea"
    return [
        {"x0": np.ascontiguousarray(xb0[b]),
         "x1": np.ascontiguousarray(xb1[b]),
         akey: np.ascontiguousarray(al[b]),
         "tmpl": tmpl, "rwt": rwt, "rb": rb, "bias": bias}
        for b in range(NCORES)
    ]


def kernel(inputs, mask, Alpha, weight_templates, routing_w, routing_b, bias,
           use_alpha):
    ua = int(np.asarray(use_alpha))
    nc = _get(ua)
    in_maps = _in_maps(dict(inputs=inputs, mask=mask, Alpha=Alpha,
                            weight_templates=weight_templates,
                            routing_w=routing_w, routing_b=routing_b,
                            bias=bias, use_alpha=use_alpha))
    res = run_bass_kernel_spmd(nc, in_maps, list(range(NCORES)))
    out = np.empty((NCORES, O, H, W), np.float32)
    plane = np.zeros((O, NPIX), np.float32)
    for b in range(NCORES):
        tiles = res.results[b]["out"].reshape(NT * 128, O)  # [pf-PT0, o]
        plane[:, PT0:PT0 + NT * 128] = tiles.T
        out[b] = plane.reshape(O, HPAD, WP)[:, 1:57, 0:W]
    return np.ascontiguousarray(out)


# revision 11
# speedup vs baseline: 1.0331x; 1.0107x over previous
"""DRConv (dynamic region-aware conv) Trainium2 kernel, v2.

Math (per batch b, all on device):
  x_se  = 0.25*sigmoid(routing_w @ mean_hw(x) + routing_b)           # [G*T]
  Z_t   = conv3x3(x, template_t)       for t in 0..T-1               # [O, H, W]
  U     = [x_se.T | 1] contracted with exp(Alpha) over g             # [T+1, P]
  out   = (sum_t Z_t * U_t) / U_T  + bias                            # [O, H, W]
which equals the reference because the template blend commutes through
the conv (blend weights and softmax probs both act per (g, pixel)).

Sharding: data-parallel over batch B=8, one batch element per core.

v2 changes vs v1:
  - x ships from host already scattered into the padded 57-pitch
    planes (bf16), so no on-device memset/copy staging.
  - Alpha ships zero-padded into the plane layout; exp(0)=1 supplies
    the pad-pixel softmax denominator for free (single ACT, no memset).
  - bf16 warmup matmuls (8) instead of 30 fp32 ones.
  - input DMA issues spread across Sync/Scalar/GpSimd queues.
  - no PE transpose: each tile's [px, o] accumulator is DMA'd straight
    to DRAM; the [px,o] -> [o,hw] flip happens in the host gather.
"""

import ml_dtypes
import numpy as np

import concourse.bass as bass
import concourse.mybir as mybir
from concourse import bacc
from concourse.tile import TileContext
from concourse.bass_utils import run_bass_kernel_spmd

# problem constants
C = 128          # in channels
O = 128          # out channels
H = W = 56
G = 8            # groups
T = 8            # num weight templates
WP = 57          # padded row width (one shared pad column)
HPAD = 58        # one pad row top and bottom
NPIX = HPAD * WP  # 3306
GUARD = 64       # front guard in the x buffer for negative conv shifts
PT0 = WP         # first pixel-tile starts at padded row 1
NT = 25          # 25 tiles of 128 px cover pf [57, 3257) > last valid 3247
AFREE = 3328     # alpha plane free size (NPIX rounded up)
XB1 = 29 * WP    # pf origin of band-B buffer
NB0 = GUARD + 32 * WP   # band-A buffer cols (pf -GUARD .. 1824)
NB1 = 30 * WP           # band-B buffer cols (pf 1653 .. 3363)
NCORES = 8
NWARM = 12       # bf16 warmup matmuls

_cache = {}


def _delta(ij):
    i, j = divmod(ij, 3)
    return (i - 1) * WP + (j - 1)


def _build(use_alpha: int):
    f32 = mybir.dt.float32
    bf16 = mybir.dt.bfloat16

    nc = bacc.Bacc("TRN2", target_bir_lowering=False, debug=False,
                   num_devices=NCORES)

    x0_d = nc.dram_tensor("x0", [C, NB0], bf16, kind="ExternalInput")
    x1_d = nc.dram_tensor("x1", [C, NB1], bf16, kind="ExternalInput")
    if use_alpha:
        alpha_d = nc.dram_tensor("alpha", [G, AFREE], f32,
                                 kind="ExternalInput")
    else:
        # hard routing: host ships ea = one_hot(mask) directly
        ea_d = nc.dram_tensor("ea", [G, AFREE], bf16, kind="ExternalInput")
    tmpl_d = nc.dram_tensor("tmpl", [9, C, T * O], bf16, kind="ExternalInput")
    rwt_d = nc.dram_tensor("rwt", [C, G * T], f32, kind="ExternalInput")
    rb_d = nc.dram_tensor("rb", [G * T], f32, kind="ExternalInput")
    bias_d = nc.dram_tensor("bias", [O], f32, kind="ExternalInput")
    out_d = nc.dram_tensor("out", [NT, 128, O], f32, kind="ExternalOutput")

    with TileContext(nc) as tc:
        with (
            tc.tile_pool(name="big", bufs=1) as big,
            tc.tile_pool(name="consts", bufs=1) as consts,
            tc.tile_pool(name="acc", bufs=3) as accp,
            tc.tile_pool(name="upool", bufs=3) as upool,
            tc.tile_pool(name="zps", bufs=3, space="PSUM") as zps,
            tc.tile_pool(name="ups", bufs=2, space="PSUM") as ups,
        ):
            # ---- PE warmup on a zeroed bf16 tile while inputs stream ----
            warmz = consts.tile([128, 512], bf16)
            nc.vector.memset(warmz[:], 0.0)
            warm = zps.tile([128, 512], f32, tag="zp0", name="warm")
            for _ in range(NWARM):
                nc.tensor.matmul(warm[:], lhsT=warmz[:, 0:128], rhs=warmz[:])

            # ---- input DMAs, issues spread across engine queues ----
            xbf0 = big.tile([C, NB0], bf16)
            nc.sync.dma_start(out=xbf0[:], in_=x0_d[:])
            xbf1 = big.tile([C, NB1], bf16)
            nc.sync.dma_start(out=xbf1[:], in_=x1_d[:])
            if use_alpha:
                ast = big.tile([G, AFREE], f32)
                nc.sync.dma_start(out=ast[:], in_=alpha_d[:])

            tbf = []
            for ij in range(9):
                tb = big.tile([C, T * O], bf16, name=f"tbf{ij}")
                eng = nc.scalar if ij < 4 else nc.gpsimd
                eng.dma_start(out=tb[:], in_=tmpl_d[ij])
                tbf.append(tb)

            rwt = consts.tile([C, G * T], f32)
            nc.gpsimd.dma_start(out=rwt[:], in_=rwt_d[:])
            rb = consts.tile([G * T, 1], f32)
            nc.gpsimd.dma_start(out=rb[:], in_=rb_d[:])
            bias_rep = consts.tile([128, O], f32)
            nc.scalar.dma_start(
                out=bias_rep[:],
                in_=bass.AP(tensor=bias_d, offset=0, ap=[[0, 128], [1, O]]),
            )

            # ---- routing: GAP -> fc -> sigmoid ----
            # band A sum covers image rows 0..30 (pads/guard are zero);
            # band B slice skips its first 3 rows (28..30, already in A)
            xsum = consts.tile([C, 1], f32)
            xsum0 = consts.tile([C, 1], f32)
            nc.vector.tensor_reduce(
                out=xsum0[:], in_=xbf0[:],
                axis=mybir.AxisListType.X, op=mybir.AluOpType.add)
            nc.vector.tensor_reduce(
                out=xsum[:], in_=xbf1[:, 3 * WP:],
                axis=mybir.AxisListType.X, op=mybir.AluOpType.add)
            nc.vector.tensor_add(xsum[:], xsum[:], xsum0[:])

            zr = ups.tile([G * T, 1], f32, tag="up")
            nc.tensor.matmul(zr[:], lhsT=rwt[:], rhs=xsum[:])
            xse = consts.tile([G * T, 1], f32)
            nc.scalar.activation(xse[:], zr[:],
                                 mybir.ActivationFunctionType.Sigmoid,
                                 bias=rb[:], scale=1.0 / (H * W))
            xse4 = consts.tile([G * T, 1], bf16)
            nc.vector.tensor_scalar_mul(xse4[:], xse[:], 2.0 / T)

            # lhsT_U [g, T+1]: cols 0..T-1 = x_se[g, t], col T = 1.0
            lhsu = consts.tile([G, T + 1], bf16)
            nc.vector.memset(lhsu[:, T:T + 1], 1.0)
            nc.sync.dma_start(out=lhsu[:, 0:T], in_=xse4[:])

            # ---- routing numerators: ea = exp(alpha), pads exp(0)=1 ----
            ea = big.tile([G, AFREE], bf16)
            if use_alpha:
                nc.scalar.activation(ea[:], ast[:],
                                     mybir.ActivationFunctionType.Exp)
            else:
                nc.sync.dma_start(out=ea[:], in_=ea_d[:])

            # ---- main loop over pixel tiles ----
            for k in range(NT):
                base = PT0 + 128 * k

                # h-outer: zp[0] finishes 9 MMs early, so the DVE mixing
                # chain (t=0..3 reads zp[0]) starts ~1.9us sooner per tile
                zp = [zps.tile([128, 512], f32, tag=f"zp{h}",
                               name=f"zp{h}_{k}")
                      for h in range(2)]
                def conv_half(h):
                    for ij in range(9):
                        if k <= 12:
                            lo = GUARD + base + _delta(ij)
                            xsl = xbf0[:, lo:lo + 128]
                        else:
                            lo = base - XB1 + _delta(ij)
                            xsl = xbf1[:, lo:lo + 128]
                        nc.tensor.matmul(
                            zp[h][:],
                            lhsT=xsl,
                            rhs=tbf[ij][:, h * 512:(h + 1) * 512],
                            start=(ij == 0), stop=(ij == 8))

                conv_half(0)
                up = ups.tile([128, T + 1], f32, tag="up")
                nc.tensor.matmul(up[:], lhsT=ea[:, base:base + 128],
                                 rhs=lhsu[:])
                conv_half(1)

                rcol = upool.tile([128, 1], f32, tag="rcol")
                nc.vector.reciprocal(rcol[:], up[:, T:T + 1])
                usb = upool.tile([128, T], f32, tag="usb")
                nc.vector.tensor_scalar_mul(usb[:], up[:, 0:T], rcol[:])

                acc = accp.tile([128, O], f32, tag="acc")
                for t in range(T):
                    h, tq = divmod(t, 4)
                    nc.vector.scalar_tensor_tensor(
                        out=acc[:],
                        in0=zp[h][:, tq * 128:(tq + 1) * 128],
                        scalar=usb[:, t:t + 1],
                        in1=bias_rep[:] if t == 0 else acc[:],
                        op0=mybir.AluOpType.mult,
                        op1=mybir.AluOpType.add)

                nc.sync.dma_start(out=out_d[k], in_=acc[:])

    nc.compile()
    return nc


def _get(use_alpha: int):
    if use_alpha not in _cache:
        _cache[use_alpha] = _build(use_alpha)
    return _cache[use_alpha]


def _in_maps(inp):
    ua = int(np.asarray(inp["use_alpha"]))
    x = np.asarray(inp["inputs"], dtype=np.float32).reshape(
        NCORES, C, H, W).astype(ml_dtypes.bfloat16)

    # band A: pf [-GUARD, 1824) = plane rows 0..31 (img rows 0..30)
    xb0 = np.zeros((NCORES, C, NB0), ml_dtypes.bfloat16)
    v0 = xb0[:, :, GUARD:].reshape(NCORES, C, 32, WP)
    v0[:, :, 1:32, 0:W] = x[:, :, 0:31, :]
    # band B: pf [1653, 3363) = plane rows 29..58 (img rows 28..55)
    xb1 = np.zeros((NCORES, C, NB1), ml_dtypes.bfloat16)
    v1 = xb1.reshape(NCORES, C, 30, WP)
    v1[:, :, 0:28, 0:W] = x[:, :, 28:56, :]

    if ua:
        # alpha scattered into the plane; zero pads -> exp=1
        al = np.zeros((NCORES, G, AFREE), np.float32)
        va = al[:, :, 0:NPIX].reshape(NCORES, G, HPAD, WP)
        va[:, :, 1:57, 0:W] = np.asarray(inp["Alpha"], dtype=np.float32)
    else:
        # hard routing: ea = one_hot(mask), pads 1.0 (any nonzero denom)
        mk = np.asarray(inp["mask"]).reshape(NCORES, H, W)
        ea = np.ones((NCORES, G, AFREE), np.float32)
        ve = ea[:, :, 0:NPIX].reshape(NCORES, G, HPAD, WP)
        ve[:, :, 1:57, 0:W] = (
            mk[:, None, :, :] == np.arange(G)[None, :, None, None])
        al = ea.astype(ml_dtypes.bfloat16)

    # [O*C*3*3, T] -> [(i,j), c, t*O + o]
    tmpl = np.asarray(inp["weight_templates"], dtype=np.float32).reshape(
        O, C, 3, 3, T).transpose(2, 3, 1, 4, 0).reshape(9, C, T * O)
    tmpl = np.ascontiguousarray(tmpl).astype(ml_dtypes.bfloat16)
    rwt = np.ascontiguousarray(
        np.asarray(inp["routing_w"], dtype=np.float32).T)
    rb = np.ascontiguousarray(np.asarray(inp["routing_b"], dtype=np.float32))
    bias = np.ascontiguousarray(np.asarray(inp["bias"], dtype=np.float32))

    akey = "alpha" if ua else "ea"
    return [
        {"x0": np.ascontiguousarray(xb0[b]),
         "x1": np.ascontiguousarray(xb1[b]),
         akey: np.ascontiguousarray(al[b]),
         "tmpl": tmpl, "rwt": rwt, "rb": rb, "bias": bias}
        for b in range(NCORES)
    ]


def kernel(inputs, mask, Alpha, weight_templates, routing_w, routing_b, bias,
           use_alpha):
    ua = int(np.asarray(use_alpha))
    nc = _get(ua)
    in_maps = _in_maps(dict(inputs=inputs, mask=mask, Alpha=Alpha,
                            weight_templates=weight_templates,
                            routing_w=routing_w, routing_b=routing_b,
                            bias=bias, use_alpha=use_alpha))
    res = run_bass_kernel_spmd(nc, in_maps, list(range(NCORES)))
    out = np.empty((NCORES, O, H, W), np.float32)
    plane = np.zeros((O, NPIX), np.float32)
    for b in range(NCORES):
        tiles = res.results[b]["out"].reshape(NT * 128, O)  # [pf-PT0, o]
        plane[:, PT0:PT0 + NT * 128] = tiles.T
        out[b] = plane.reshape(O, HPAD, WP)[:, 1:57, 0:W]
    return np.ascontiguousarray(out)
